# revision 26
# baseline (speedup 1.0000x reference)
"""Trainium2 Bass kernel for the NiN-Conv2D problem.

Network: per-pixel MLP over 7x7x3 patches, independent per filter f:
  h0 = relu(P @ W0[:,:,f] + b0)   (147 -> 32)
  h1 = relu(h0 @ W1[:,:,f] + b1)  (32 -> 16)
  out = relu(h1 @ W2[:,:,f] + b2) (16 -> 1)
for B=32, H=W=32, F=128.

Strategy: data-parallel over batch across 8 NeuronCores (4 images each).
On each core everything runs feature-major: activations live as (d*f on
partitions, pixels on free dim), weights are the stationary matmul operand.

This version is engineered around the real bottleneck: the PSUM->SBUF
relu+cast traffic on ScalarE/VectorE (fp32 PSUM reads are 1 elem/cycle/
partition on both engines, and each op pays ~250ns of fixed overhead).
Changes vs the earlier baseline:

  * Supertiles of 1024 pixels (2 x 512-px halves). L1/out relus span both
    halves, so every activation op is FD=1024..2048 instead of 512..1024,
    halving the per-op overhead count while keeping per-partition biases
    legal (bias varies per partition, constant along pixels).
  * One (128, 4096) fp32 PSUM tensor managed as a ring of 8 bank slots
    (aligned: quads take 4 contiguous slots, L1 pairs / L2 take 2), so a
    single dense relu op covers a whole quad (FD=2048) or a whole L1 pair
    (FD=1024). Tile's subtile dependency tracking provides the hazards.
  * Relu ops are greedily balanced across ScalarE/VectorE with a cost
    model (scalar: 253+FD/1.2 ns, vector: 220+FD/0.96 ns).
  * DMA staging strictly in first-use order so the first quad's weights
    and patches land first.

PE layout (unchanged): L0 chunk1 = full-array K=128 matmuls, chunk2
(K rows 128..146 + b0 ones-row) = 4-way row-tiled concurrent matmuls
sharing one stream; L1 = 2-way col-tiled block-diag W1; L2 = 4-way
col-tiled accumulating block-diag W2. All operands bf16, fp32 PSUM.
"""
import numpy as np
import ml_dtypes

import concourse.bass as bass
import concourse.mybir as mybir
from concourse import bacc, tile
from concourse import bass_utils
from concourse.bass import ts

KH, KW = 7, 7
B, H, W, C, F = 32, 32, 32, 3, 128
K, D0, D1 = 147, 32, 16
NCORES = 8
BPC = B // NCORES            # 4 images per core
NPIX = BPC * H * W           # 4096 pixels per core
PTILE = 512                  # L0 subtile (one PSUM bank of fp32)
NT = NPIX // PTILE           # 8 subtiles
PST = 1024                   # supertile pixels (2 subtiles)
NST = NPIX // PST            # 4 supertiles

BF16 = mybir.dt.bfloat16
F32 = mybir.dt.float32
NPBF16 = ml_dtypes.bfloat16


# ----------------------------------------------------------------------------
# host-side packing (layout only)
# ----------------------------------------------------------------------------

def _pack_weights(w0, b0, w1, b1, w2, b2):
    """Shared (core-independent) weight/bias packing. Returns dict of np arrays."""
    w0 = np.asarray(w0, np.float32)
    w1 = np.asarray(w1, np.float32)
    w2 = np.asarray(w2, np.float32)
    b0 = np.asarray(b0, np.float32)
    b1 = np.asarray(b1, np.float32)
    b2 = np.asarray(b2, np.float32)

    w0a = np.empty((128, 32, 128), np.float32)   # [k, group, m=fl*32+d]
    # chunk2 (K rows 128..146 + bias row) packed for 4-way row-tiled
    # concurrency: group g lives at partitions 32*(g%4)+k, cols g*128+m.
    # Row 32*(g%4)+19 carries b0 (the patch tile has ones there), so the
    # PSUM result already includes the bias and the relu op needs none.
    w0b = np.zeros((128, 32, 128), np.float32)
    for g in range(32):
        m = w0[:, :, 4 * g:4 * g + 4].transpose(0, 2, 1).reshape(K, 128)
        w0a[:, g, :] = m[:128]
        r = g % 4
        w0b[32 * r:32 * r + 19, g, :] = m[128:]
        w0b[32 * r + 19, g, :] = b0[:, 4 * g:4 * g + 4].T.reshape(128)

    w1bd = np.zeros((128, 32, 64), np.float32)   # [k=fl*32+d0, g, m=fl*16+d1]
    b1s = np.empty((128, 16), np.float32)
    for g in range(32):
        for fl in range(4):
            f = 4 * g + fl
            w1bd[fl * 32:(fl + 1) * 32, g, fl * 16:(fl + 1) * 16] = w1[:, :, f]
    for p in range(16):
        for half in range(2):
            g = 2 * p + half
            b1s[half * 64:(half + 1) * 64, p] = b1[:, 4 * g:4 * g + 4].T.reshape(64)

    w2bd = np.zeros((128, 16, 32), np.float32)   # [k=half*64+fl*16+d1, pair, col]
    for p in range(16):
        for half in range(2):
            for fl in range(4):
                f = 8 * p + half * 4 + fl
                col = f - 32 * (p // 4)
                w2bd[half * 64 + fl * 16:half * 64 + (fl + 1) * 16, p, col] = w2[:, 0, f]
    b2s = b2.reshape(128, 1).astype(np.float32)

    return {
        "w0a": w0a.reshape(128, 4096).astype(NPBF16),
        "w0b": w0b.reshape(128, 4096).astype(NPBF16),
        "w1bd": w1bd.reshape(128, 2048).astype(NPBF16),
        "w2bd": w2bd.reshape(128, 512).astype(NPBF16),
        "b1s": b1s, "b2s": b2s,
    }


def _im2col_T(x_core):
    """x_core (4,32,32,3) fp32 -> PT (147, 4096) with k=(kh*7+kw)*3+c."""
    xp = np.pad(np.asarray(x_core, np.float32), ((0, 0), (3, 3), (3, 3), (0, 0)))
    PT = np.empty((K, NPIX), np.float32)
    for kh in range(KH):
        for kw in range(KW):
            blk = xp[:, kh:kh + H, kw:kw + W, :]
            t = kh * 7 + kw
            PT[t * 3:t * 3 + 3] = blk.transpose(3, 0, 1, 2).reshape(3, NPIX)
    return PT


# ----------------------------------------------------------------------------
# device kernel
# ----------------------------------------------------------------------------

def _body(tc):
    nc = tc.nc
    Relu = mybir.ActivationFunctionType.Relu
    Add, Max = mybir.AluOpType.add, mybir.AluOpType.max

    pt1 = nc.dram_tensor("pt1", [128, NPIX], BF16, kind="ExternalInput").ap()
    pt2 = nc.dram_tensor("pt2", [128, NPIX], BF16, kind="ExternalInput").ap()
    w0a = nc.dram_tensor("w0a", [128, 4096], BF16, kind="ExternalInput").ap()
    w0b = nc.dram_tensor("w0b", [128, 4096], BF16, kind="ExternalInput").ap()
    w1bd = nc.dram_tensor("w1bd", [128, 2048], BF16, kind="ExternalInput").ap()
    w2bd = nc.dram_tensor("w2bd", [128, 512], BF16, kind="ExternalInput").ap()
    b1d = nc.dram_tensor("b1s", [128, 16], F32, kind="ExternalInput").ap()
    b2d = nc.dram_tensor("b2s", [128, 1], F32, kind="ExternalInput").ap()
    out = nc.dram_tensor("out", [128, NPIX], F32, kind="ExternalOutput").ap()

    with (
        tc.tile_pool(name="consts", bufs=1) as cpool,
        tc.tile_pool(name="h0", bufs=36) as h0pool,
        tc.tile_pool(name="h1", bufs=18) as h1pool,
        tc.tile_pool(name="outs", bufs=4) as opool,
        tc.tile_pool(name="ps", bufs=1, space="PSUM") as pspool,
    ):
        psum = pspool.tile([128, 4096], F32, tag="ps", name="psum_ring")

        # ---- HAM warmup: one accumulation chain of dummy matmuls over
        # memset scratch keeps the PE busy through the ~11us DMA boot, so
        # the first real quad runs at the warm 2.4GHz clock instead of the
        # cold 1.2GHz (the HAM gate needs ~3.4us of sustained activity and
        # re-throttles only after ~3.4us idle). The dummies accumulate
        # garbage into ring slot 7, which the first real user (quad 1's
        # psB, start=True) clears.
        wscr = cpool.tile([128, 128], BF16, tag="wscr", name="wscr")
        nc.gpsimd.memset(wscr[:], 0)
        pscr = cpool.tile([128, 512], BF16, tag="pscr", name="pscr")
        nc.gpsimd.memset(pscr[:], 0)
        NDUMMY = 30
        for i in range(NDUMMY):
            nc.tensor.matmul(psum[:, 3584:4096], wscr[:], pscr[:],
                             start=(i == 0), stop=(i == NDUMMY - 1))

        # ---- DMA staging in first-use order. The first quad needs only
        # w0a/w0b groups 0-3 and patch tile 0, so those go first; the HAM
        # clock then ramps on real matmul work instead of DMA waits.
        was, wbs, pt1s, pt2s = [], [], [], []

        def load_w_chunk(i):
            wa = cpool.tile([128, 512], BF16, tag=f"w0a{i}", name=f"w0a{i}")
            nc.sync.dma_start(wa[:], w0a[:, ts(i, 512)])
            was.append(wa)
            wb = cpool.tile([128, 512], BF16, tag=f"w0b{i}", name=f"w0b{i}")
            nc.sync.dma_start(wb[:], w0b[:, ts(i, 512)])
            wbs.append(wb)

        def load_pt(t):
            p1 = cpool.tile([128, PTILE], BF16, tag=f"pt1_{t}", name=f"pt1_{t}")
            nc.sync.dma_start(p1[:], pt1[:, ts(t, PTILE)])
            pt1s.append(p1)
            p2 = cpool.tile([128, PTILE], BF16, tag=f"pt2_{t}", name=f"pt2_{t}")
            nc.sync.dma_start(p2[:], pt2[:, ts(t, PTILE)])
            pt2s.append(p2)

        # First quad gates only on w0a group 0 + patch tile 0, so chunk 0
        # is loaded group-by-group (32KB pieces) with pt1 t0 right behind:
        # the first matmul issues after ~160KB instead of ~640KB.
        wa0 = cpool.tile([128, 512], BF16, tag="w0a0", name="w0a0")
        nc.sync.dma_start(wa0[:, ts(0, 128)], w0a[:, ts(0, 128)])
        p1 = cpool.tile([128, PTILE], BF16, tag="pt1_0", name="pt1_0")
        nc.sync.dma_start(p1[:], pt1[:, ts(0, PTILE)])
        pt1s.append(p1)
        for g in range(1, 4):
            nc.sync.dma_start(wa0[:, ts(g, 128)], w0a[:, ts(g, 128)])
        was.append(wa0)
        wb0 = cpool.tile([128, 512], BF16, tag="w0b0", name="w0b0")
        nc.sync.dma_start(wb0[:], w0b[:, ts(0, 512)])
        wbs.append(wb0)
        p2 = cpool.tile([128, PTILE], BF16, tag="pt2_0", name="pt2_0")
        nc.sync.dma_start(p2[:], pt2[:, ts(0, PTILE)])
        pt2s.append(p2)
        load_w_chunk(1)
        load_pt(1)
        for i in range(2, 8):    # groups 8-31
            load_w_chunk(i)
        w1s = cpool.tile([128, 2048], BF16, tag="w1", name="w1s")
        nc.sync.dma_start(w1s[:], w1bd)
        w2s = cpool.tile([128, 512], BF16, tag="w2", name="w2s")
        nc.sync.dma_start(w2s[:], w2bd)
        b1s = cpool.tile([128, 16], F32, tag="b1", name="b1s_t")
        nc.sync.dma_start(b1s[:], b1d)
        b2s = cpool.tile([128, 1], F32, tag="b2", name="b2s_t")
        nc.sync.dma_start(b2s[:], b2d)
        for t in range(2, NT):
            load_pt(t)

        # ---- PSUM ring allocator: 8 bank-slots of 512 fp32 columns.
        # Aligned allocation (quads 4 slots, pairs/L2 2) keeps every relu
        # source a contiguous slice and every matmul dst inside one bank.
        cur = [0]

        def ralloc(n):
            c = (cur[0] + n - 1) // n * n
            cur[0] = c + n
            return (c % 8) * 512

        # ---- greedy ScalarE/VectorE balancing for PSUM->SBUF relu ops
        ecost = [0.0, 0.0]       # scalar, vector

        def relu_op(dst, src, bias=None, engine=None):
            fd = src.shape[-1]
            cs = 253.0 + fd / 1.2
            cv = 220.0 + fd / 0.96
            if engine is None:
                engine = 0 if ecost[0] + cs <= ecost[1] + cv else 1
            if engine == 0:
                ecost[0] += cs
                nc.scalar.activation(dst, src, Relu,
                                     bias=bias if bias is not None else 0.0)
            else:
                ecost[1] += cv
                if bias is None:
                    nc.vector.tensor_scalar_max(dst, src, 0.0)
                else:
                    nc.vector.tensor_scalar(dst, src, bias, 0.0, Add, Max)

        h0t = {}                 # (subtile, duo) -> (128, 1024) bf16
        for st in range(NST):
            # ---- L0: both pixel halves, in quads of 4 filter-groups:
            # 4 full-K chunk1 matmuls open the accumulation groups (they
            # pipeline back-to-back at ~215ns), then ONE 4-way row-tiled
            # chunk2 stream closes all four. The relu is split psA/psB
            # (FD=1024 each) on opposite engines, so the ring recurrence
            # (2 quads deep) stays under the PE's own issue rate.
            for h in (0, 1):
                t = 2 * st + h
                for q in range(8):
                    c = ralloc(4)
                    for r in range(4):
                        g = 4 * q + r
                        nc.tensor.matmul(psum[:, c + 512 * r:c + 512 * (r + 1)],
                                         was[g // 4][:, ts(g % 4, 128)],
                                         pt1s[t][:], start=True, stop=False)
                    for r in range(4):
                        g = 4 * q + r
                        nc.tensor.matmul(psum[:, c + 512 * r:c + 512 * (r + 1)],
                                         wbs[g // 4][32 * r:32 * r + 20, ts(g % 4, 128)],
                                         pt2s[t][32 * r:32 * r + 20, :],
                                         start=False, stop=True,
                                         tile_position=(32 * r, 0))
                    for half in (0, 1):       # psA / psB -> duo tiles 2q, 2q+1
                        h0 = h0pool.tile([128, 1024], BF16, tag="h0",
                                         name=f"h0_{t}_{2 * q + half}")
                        relu_op(h0[:], psum[:, c + 1024 * half:c + 1024 * (half + 1)],
                                engine=(q + half) % 2)
                        h0t[(t, 2 * q + half)] = h0
            # ---- L1: 16 pairs, each spanning both pixel halves so the
            # (128,1) b1 bias is legal on a single merged relu op.
            h1t = []
            for p in range(16):
                cp = ralloc(2)
                for hh in (0, 1):
                    src = h0t[(2 * st + hh, p)]   # duo p == groups 2p,2p+1
                    dst = psum[:, cp + 512 * hh:cp + 512 * (hh + 1)]
                    nc.tensor.matmul(dst[0:64, :],
                                     w1s[:, ts(2 * p, 64)],
                                     src[:, ts(0, 512)],
                                     start=True, stop=True)
                    nc.tensor.matmul(dst[64:128, :],
                                     w1s[:, ts(2 * p + 1, 64)],
                                     src[:, ts(1, 512)],
                                     start=True, stop=True)
                h1 = h1pool.tile([128, PST], BF16, tag="h1",
                                 name=f"h1_{st}_{p}")
                relu_op(h1[:], psum[:, cp:cp + 1024],
                        bias=b1s[:, p:p + 1])
                h1t.append(h1)
            # ---- L2: per pixel half, 4 col-groups x 4 accumulating
            # matmuls (q-major: the 4 col groups run concurrently), with
            # each half's relu + output DMA issued before the other
            # half's matmuls so the tail DMA overlaps L2 compute. b2 is
            # per-partition, so any pixel range is legal for the relu.
            c2 = ralloc(2)
            for hh in (0, 1):
                dst = psum[:, c2 + 512 * hh:c2 + 512 * (hh + 1)]
                for qq in range(4):
                    for jj in range(4):
                        p = 4 * jj + qq
                        nc.tensor.matmul(dst[32 * jj:32 * jj + 32, :],
                                         w2s[:, ts(p, 32)],
                                         h1t[p][:, ts(hh, 512)],
                                         start=(qq == 0), stop=(qq == 3),
                                         tile_position=(0, 32 * jj))
                ot = opool.tile([128, PTILE], F32, tag="o",
                                name=f"o_{st}_{hh}")
                relu_op(ot[:], psum[:, c2 + 512 * hh:c2 + 512 * (hh + 1)],
                        bias=b2s[:, 0:1])
                nc.sync.dma_start(out[:, st * PST + 512 * hh:
                                      st * PST + 512 * (hh + 1)], ot[:])


_COMPILED = None


def _get_compiled():
    global _COMPILED
    if _COMPILED is None:
        import time as _time
        t0 = _time.time()
        nc = bacc.Bacc("TRN2", target_bir_lowering=False, debug=False,
                       num_devices=NCORES)
        with tile.TileContext(nc) as tc:
            _body(tc)
        t1 = _time.time()
        nc.compile()
        t2 = _time.time()
        print(f"[kernel] tile build+schedule {t1 - t0:.1f}s, bacc compile {t2 - t1:.1f}s",
              flush=True)
        _COMPILED = nc
    return _COMPILED


# ----------------------------------------------------------------------------
# public entry point
# ----------------------------------------------------------------------------

def kernel(x, w0, b0, w1, b1, w2, b2, _trace=False):
    x = np.asarray(x, np.float32)
    shared = _pack_weights(w0, b0, w1, b1, w2, b2)

    in_maps = []
    for k in range(NCORES):
        PT = _im2col_T(x[BPC * k:BPC * (k + 1)])
        m = dict(shared)
        m["pt1"] = PT[:128].astype(NPBF16)
        # chunk2 rows replicated at partitions 32r (4-way row tiling),
        # with a ones row at 32r+19 that carries b0 through the matmul
        pt2 = np.zeros((128, NPIX), np.float32)
        for r in range(4):
            pt2[32 * r:32 * r + 19] = PT[128:]
            pt2[32 * r + 19] = 1.0
        m["pt2"] = pt2.astype(NPBF16)
        in_maps.append(m)

    import time as _time
    nc = _get_compiled()
    t0 = _time.time()
    res = bass_utils.run_bass_kernel_spmd(
        nc, in_maps, core_ids=list(range(NCORES)), trace=_trace)
    print(f"[kernel] run_bass_kernel_spmd {_time.time() - t0:.1f}s", flush=True)

    outs = []
    for k in range(NCORES):
        oc = np.asarray(res.results[k]["out"], np.float32)   # (128, 4096)
        outs.append(oc.reshape(F, BPC, H, W).transpose(1, 2, 3, 0))
    full = np.concatenate(outs, axis=0).astype(np.float32)
    if _trace:
        return full, res
    return full
